# revision 2
# baseline (speedup 1.0000x reference)
"""ContentAttention kernel for 8 Trainium2 NeuronCores.

Computation (per batch b):
    h_att  = h[b] @ W_h2att + b_h2att                  # [512]
    e      = tanh(p_att_feats[b] + h_att)              # [1024, 512]
    scores = e @ w_alpha (+ b_alpha, dropped: softmax shift-invariant)
    w      = softmax(scores)                           # [1024]
    out[b] = w @ att_feats[b]                          # [1024]

Sharding: data-parallel over batch B=128 -> 16 batches/core x 8 cores.
Params are tiny and replicated. ~96MB of HBM reads per core (p_att_feats
32MB + att_feats 64MB) makes this DMA-bound.

Per-core design:
  - p slabs [128(regions), 4, 512(d)] (4 region-chunks per DMA);
    h_att broadcast across partitions via stride-0 DMA; in-place
    DVE add -> ACT tanh -> per-chunk DVE scalar_tensor_tensor
    (fused *w_alpha multiply + free-dim sum) giving [128,1] score cols.
  - softmax over 1024 scores held as [128, 8]: cross-partition max/sum
    via gpsimd partition_all_reduce (keeps PE free for matmuls).
  - weighted sum over regions on PE: lhsT = weight column [128,1],
    rhs = att_feats slab [128, 2, 1024] (natural layout), accumulated
    over 8 region chunks into PSUM [1, 512] x2.
  - DMA issue spread across rings: att_feats on sync (HWDGE),
    p and outputs on scalar (HWDGE), broadcasts/consts on gpsimd.
"""

import numpy as np

B, R, K_H, D, F = 128, 1024, 1024, 512, 1024
N_CORES = 8
BPC = B // N_CORES  # batches per core
RC = R // 128  # region chunks (8)
PC = 4  # p-chunks per slab
AC = 2  # a-chunks per slab

_cached = {}


def _build_program():
    from contextlib import ExitStack

    import concourse.bass as bass
    import concourse.bass_isa as bass_isa
    import concourse.tile as tile
    from concourse import bacc, mybir

    f32 = mybir.dt.float32
    AF = mybir.ActivationFunctionType
    ALU = mybir.AluOpType
    AX = mybir.AxisListType

    def bcast_p(row_ap, parts):
        # replicate a [1, n] DRAM row across `parts` partitions
        return bass.AP(
            tensor=row_ap.tensor,
            offset=row_ap.offset,
            ap=[[0, parts], list(row_ap.ap[-1])],
        )

    def bcast_mid(t, c):
        # [128, D] sbuf tile -> [128, c, D] view with 0-stride middle dim
        return bass.AP(
            tensor=t.tensor,
            offset=t.offset,
            ap=[list(t.ap[0]), [0, c], list(t.ap[1])],
        )

    nc = bacc.Bacc("TRN2", target_bir_lowering=False, debug=False)
    h_ap = nc.dram_tensor("h", [BPC, K_H], f32, kind="ExternalInput").ap()
    att_ap = nc.dram_tensor("att", [BPC, R, F], f32, kind="ExternalInput").ap()
    p_ap = nc.dram_tensor("p", [BPC, R, D], f32, kind="ExternalInput").ap()
    w_ap = nc.dram_tensor("w_h2att", [K_H, D], f32, kind="ExternalInput").ap()
    b2_ap = nc.dram_tensor("b_h2att", [1, D], f32, kind="ExternalInput").ap()
    wa_ap = nc.dram_tensor("w_alpha", [1, D], f32, kind="ExternalInput").ap()
    out_ap = nc.dram_tensor("out", [BPC, F], f32, kind="ExternalOutput").ap()

    with tile.TileContext(nc) as tc, ExitStack() as ctx:
        consts = ctx.enter_context(tc.tile_pool(name="consts", bufs=1))
        wpool = ctx.enter_context(tc.tile_pool(name="wpool", bufs=1))
        ppool = ctx.enter_context(tc.tile_pool(name="ppool", bufs=6))
        apool = ctx.enter_context(tc.tile_pool(name="apool", bufs=10))
        hbpool = ctx.enter_context(tc.tile_pool(name="hbpool", bufs=3))
        spool = ctx.enter_context(tc.tile_pool(name="spool", bufs=4))
        outp = ctx.enter_context(tc.tile_pool(name="outp", bufs=3))
        dramp = ctx.enter_context(tc.tile_pool(name="dram", bufs=1, space="DRAM"))
        ps_mm = ctx.enter_context(tc.tile_pool(name="ps_mm", bufs=3, space="PSUM"))
        ps_mic = ctx.enter_context(tc.tile_pool(name="ps_mic", bufs=1, space="PSUM"))

        walpha_bc = consts.tile([128, D], f32)
        nc.sync.dma_start(out=walpha_bc, in_=bcast_p(wa_ap, 128))

        # ---- phase 0: h_att_plus = h @ W + b_h2att -> DRAM scratch
        KC = K_H // 128
        w_all = wpool.tile([128, KC, D], f32, tag="w")
        nc.sync.dma_start(
            out=w_all, in_=w_ap.rearrange("(kc p) d -> p kc d", p=128)
        )
        h_nat = wpool.tile([BPC, K_H], f32, tag="hnat")
        nc.sync.dma_start(out=h_nat, in_=h_ap)
        ident16 = consts.tile([BPC, BPC], f32)
        from concourse.masks import make_identity

        make_identity(nc, ident16)
        # transpose h on PE: [16, 128] chunks -> [128, 16]
        hT_all = wpool.tile([128, KC, BPC], f32, tag="hT")
        for kc in range(KC):
            tr_ps = ps_mm.tile([128, BPC], f32, tag="ps0")
            nc.tensor.transpose(
                tr_ps, h_nat[:, kc * 128 : (kc + 1) * 128], ident16
            )
            nc.scalar.copy(hT_all[:, kc, :], tr_ps)
        ps_hatt = ps_mic.tile([BPC, D], f32, tag="mic")
        for kc in range(KC):
            nc.tensor.matmul(
                ps_hatt,
                lhsT=hT_all[:, kc, :],
                rhs=w_all[:, kc, :],
                start=(kc == 0),
                stop=(kc == KC - 1),
            )
        b2_bc = consts.tile([BPC, D], f32)
        nc.sync.dma_start(out=b2_bc, in_=bcast_p(b2_ap, BPC))
        hatt = consts.tile([BPC, D], f32)
        nc.vector.tensor_add(hatt, ps_hatt, b2_bc)
        hatt_dram = dramp.tile([BPC, D], f32)
        nc.scalar.dma_start(out=hatt_dram, in_=hatt)

        # ---- main loop over this core's batches
        for b in range(BPC):
            hb = hbpool.tile([128, D], f32)
            nc.gpsimd.dma_start(out=hb, in_=bcast_p(hatt_dram[b : b + 1, :], 128))
            p_view = p_ap[b].rearrange("(j c p) d -> j p c d", c=PC, p=128)
            scores = spool.tile([128, RC], f32, tag="scores")
            for j in range(R // (128 * PC)):  # 2 slabs
                p_t = ppool.tile([128, PC, D], f32, tag="p")
                nc.scalar.dma_start(out=p_t, in_=p_view[j])
                nc.vector.tensor_add(p_t, p_t, bcast_mid(hb, PC))
                nc.scalar.activation(p_t, p_t, AF.Tanh)
                for c in range(PC):
                    rc = j * PC + c
                    # out = (e * 1.0) * w_alpha; accum_out = sum -> score col
                    nc.vector.scalar_tensor_tensor(
                        out=p_t[:, c, :],
                        in0=p_t[:, c, :],
                        scalar=1.0,
                        in1=walpha_bc,
                        op0=ALU.mult,
                        op1=ALU.mult,
                        accum_out=scores[:, rc : rc + 1],
                    )

            # softmax over the 1024 scores laid out as [128 partitions, RC]
            m1 = spool.tile([128, 1], f32, tag="m1")
            nc.vector.reduce_max(m1, scores, axis=AX.X)
            mx = spool.tile([128, 1], f32, tag="mx")
            nc.gpsimd.partition_all_reduce(
                mx, m1, channels=128, reduce_op=bass_isa.ReduceOp.max
            )
            nm = spool.tile([128, 1], f32, tag="nm")
            nc.vector.tensor_scalar_mul(nm, mx, -1.0)
            expb = spool.tile([128, RC], f32, tag="expb")
            nc.scalar.activation(expb, scores, AF.Exp, bias=nm, scale=1.0)
            s1 = spool.tile([128, 1], f32, tag="s1")
            nc.vector.reduce_sum(s1, expb, axis=AX.X)
            sm = spool.tile([128, 1], f32, tag="sm")
            nc.gpsimd.partition_all_reduce(
                sm, s1, channels=128, reduce_op=bass_isa.ReduceOp.add
            )
            rec = spool.tile([128, 1], f32, tag="rec")
            nc.vector.reciprocal(rec, sm)
            wgt = spool.tile([128, RC], f32, tag="wgt")
            nc.vector.tensor_scalar_mul(wgt, expb, rec)

            # ---- phase 2: out[b] = weight @ att_feats[b]
            a_view = att_ap[b].rearrange("(j c p) f -> j p c f", c=AC, p=128)
            ps0 = ps_mm.tile([1, 512], f32, tag="ps0")
            ps1 = ps_mm.tile([1, 512], f32, tag="ps1")
            for j in range(R // (128 * AC)):  # 4 slabs
                a_t = apool.tile([128, AC, F], f32, tag="a")
                nc.sync.dma_start(out=a_t, in_=a_view[j])
                for c in range(AC):
                    rc = j * AC + c
                    nc.tensor.matmul(
                        ps0,
                        lhsT=wgt[:, rc : rc + 1],
                        rhs=a_t[:, c, 0:512],
                        start=(rc == 0),
                        stop=(rc == RC - 1),
                    )
                    nc.tensor.matmul(
                        ps1,
                        lhsT=wgt[:, rc : rc + 1],
                        rhs=a_t[:, c, 512:1024],
                        start=(rc == 0),
                        stop=(rc == RC - 1),
                    )
            ob = outp.tile([1, F], f32)
            nc.scalar.copy(ob[:, 0:512], ps0)
            nc.scalar.copy(ob[:, 512:1024], ps1)
            nc.scalar.dma_start(out=out_ap[b : b + 1, :], in_=ob)

    nc.compile()
    return nc


def _get_program():
    if "nc" not in _cached:
        _cached["nc"] = _build_program()
    return _cached["nc"]


def _make_in_maps(inputs):
    h = np.ascontiguousarray(np.asarray(inputs["h"], dtype=np.float32))
    att = np.ascontiguousarray(np.asarray(inputs["att_feats"], dtype=np.float32))
    p = np.ascontiguousarray(np.asarray(inputs["p_att_feats"], dtype=np.float32))
    W = np.ascontiguousarray(np.asarray(inputs["W_h2att"], dtype=np.float32))
    b2 = np.ascontiguousarray(
        np.asarray(inputs["b_h2att"], dtype=np.float32).reshape(1, D)
    )
    wa = np.ascontiguousarray(
        np.asarray(inputs["w_alpha"], dtype=np.float32).reshape(1, D)
    )
    # b_alpha is a scalar added to every score; softmax is shift-invariant.
    in_maps = []
    for c in range(N_CORES):
        lo, hi = c * BPC, (c + 1) * BPC
        in_maps.append(
            {
                "h": h[lo:hi],
                "att": att[lo:hi],
                "p": p[lo:hi],
                "w_h2att": W,
                "b_h2att": b2,
                "w_alpha": wa,
            }
        )
    return in_maps


def kernel(**inputs) -> np.ndarray:
    from concourse.bass_utils import run_bass_kernel_spmd

    nc = _get_program()
    in_maps = _make_in_maps(inputs)
    res = run_bass_kernel_spmd(nc, in_maps, list(range(N_CORES)))
    out = np.concatenate([res.results[c]["out"] for c in range(N_CORES)], axis=0)
    return out.astype(np.float32)



# revision 9
# speedup vs baseline: 1.6887x; 1.6887x over previous
"""ContentAttention kernel for 8 Trainium2 NeuronCores.

Computation (per batch b):
    h_att  = h[b] @ W_h2att + b_h2att                  # [512]
    e      = tanh(p_att_feats[b] + h_att)              # [1024, 512]
    scores = e @ w_alpha (+ b_alpha, dropped: softmax shift-invariant)
    w      = softmax(scores)                           # [1024]
    out[b] = w @ att_feats[b]                          # [1024]

Sharding: data-parallel over batch B=128 -> 16 batches/core x 8 cores.
Params are tiny and replicated.

The kernel is HBM-DMA-bound, so the two big streams (p_att_feats,
att_feats) are cast to bf16 on the host before upload: 48MB of HBM
reads per core instead of 96MB (output rel-err ~2.6e-3, well under the
2e-2 gate; accumulations stay fp32 in PSUM / DVE accum registers).

Per-core design:
  - regions are mapped partition-major (r = p*8 + j) so each batch's
    slab is one contiguous 8KB (p) / 16KB (att) run per partition ->
    one DMA descriptor per partition per slab. Softmax and the
    weighted sum are invariant to any consistent region permutation.
  - p slab [128, 8, 512] bf16 (1MB, one DMA on the ACT HWDGE ring);
    att slab [128, 8, 1024] bf16 (2MB, one DMA on the SP HWDGE ring).
  - h_att[b] broadcast across partitions via a PE ones-matmul into
    PSUM + ACT copy to bf16 SBUF (no HBM-amplified stride-0 DMA).
  - DVE add -> ACT tanh -> per-chunk DVE scalar_tensor_tensor
    (fused *w_alpha multiply + free-dim sum) giving [128,1] score cols.
  - softmax over 1024 scores held as [128, 8]: cross-partition max/sum
    via gpsimd partition_all_reduce; weights downcast to bf16.
  - weighted sum over regions on PE: lhsT = weight column [128,1] bf16,
    rhs = att slab chunk [128, 512] bf16, accumulated over 8 region
    chunks into PSUM [1, 512] x2.
"""

import numpy as np

B, R, K_H, D, F = 128, 1024, 1024, 512, 1024
N_CORES = 8
BPC = B // N_CORES  # batches per core
RC = R // 128  # region chunks per batch (r = p*RC + j)
KC = K_H // 128

_cached = {}


def _build_program():
    from contextlib import ExitStack

    import concourse.bass as bass
    import concourse.bass_isa as bass_isa
    import concourse.tile as tile
    from concourse import bacc, mybir

    f32 = mybir.dt.float32
    bf16 = mybir.dt.bfloat16
    AF = mybir.ActivationFunctionType
    ALU = mybir.AluOpType
    AX = mybir.AxisListType

    def bcast_p(row_ap, parts):
        # replicate a [1, n] DRAM row across `parts` partitions
        return bass.AP(
            tensor=row_ap.tensor,
            offset=row_ap.offset,
            ap=[[0, parts], list(row_ap.ap[-1])],
        )

    def bcast_mid(t, c):
        # [128, D] sbuf tile -> [128, c, D] view with 0-stride middle dim
        return bass.AP(
            tensor=t.tensor,
            offset=t.offset,
            ap=[list(t.ap[0]), [0, c], list(t.ap[1])],
        )

    nc = bacc.Bacc("TRN2", target_bir_lowering=False, debug=False)
    oh_ap = nc.dram_tensor("onehots", [BPC, BPC * 128], f32, kind="ExternalInput").ap()
    h_ap = nc.dram_tensor("h", [BPC, K_H], f32, kind="ExternalInput").ap()
    att_ap = nc.dram_tensor("att", [BPC, R, F], bf16, kind="ExternalInput").ap()
    p_ap = nc.dram_tensor("p", [BPC, R, D], bf16, kind="ExternalInput").ap()
    w_ap = nc.dram_tensor("w_h2att", [K_H, D], f32, kind="ExternalInput").ap()
    b2_ap = nc.dram_tensor("b_h2att", [1, D], f32, kind="ExternalInput").ap()
    wa_ap = nc.dram_tensor("w_alpha", [1, D], bf16, kind="ExternalInput").ap()
    out_ap = nc.dram_tensor("out", [BPC, F], f32, kind="ExternalOutput").ap()

    with tile.TileContext(nc) as tc, ExitStack() as ctx:
        consts = ctx.enter_context(tc.tile_pool(name="consts", bufs=1))
        wpool = ctx.enter_context(tc.tile_pool(name="wpool", bufs=1))
        ppool = ctx.enter_context(tc.tile_pool(name="ppool", bufs=3))
        apool = ctx.enter_context(tc.tile_pool(name="apool", bufs=3))
        hbpool = ctx.enter_context(tc.tile_pool(name="hbpool", bufs=2))
        spool = ctx.enter_context(tc.tile_pool(name="spool", bufs=4))
        outp = ctx.enter_context(tc.tile_pool(name="outp", bufs=3))
        ps_mm = ctx.enter_context(tc.tile_pool(name="ps_mm", bufs=2, space="PSUM"))
        ps_bc = ctx.enter_context(tc.tile_pool(name="ps_bc", bufs=2, space="PSUM"))
        ps_mic = ctx.enter_context(tc.tile_pool(name="ps_mic", bufs=1, space="PSUM"))

        walpha_bc = consts.tile([128, D], bf16)
        nc.sync.dma_start(out=walpha_bc, in_=bcast_p(wa_ap, 128))

        # ---- phase 0: h_att = h @ W + b_h2att, kept in SBUF
        w_all = wpool.tile([128, KC, D], f32, tag="w")
        nc.sync.dma_start(
            out=w_all, in_=w_ap.rearrange("(kc p) d -> p kc d", p=128)
        )
        h_nat = wpool.tile([BPC, K_H], f32, tag="hnat")
        nc.scalar.dma_start(out=h_nat, in_=h_ap)
        ident16 = consts.tile([BPC, BPC], f32)
        from concourse.masks import make_identity

        make_identity(nc, ident16)
        # onehots[:, b*128:(b+1)*128] is delta_{k,b} as a [16, 128] lhsT:
        # matmul against hatt broadcasts row b across all 128 partitions.
        onehots = consts.tile([BPC, BPC * 128], f32)
        nc.gpsimd.dma_start(out=onehots, in_=oh_ap)
        # transpose h on PE: [16, 128] chunks -> [128, 16]
        hT_all = wpool.tile([128, KC, BPC], f32, tag="hT")
        for kc in range(KC):
            tr_ps = ps_mm.tile([128, BPC], f32, tag="ps0")
            nc.tensor.transpose(
                tr_ps, h_nat[:, kc * 128 : (kc + 1) * 128], ident16
            )
            nc.scalar.copy(hT_all[:, kc, :], tr_ps)
        ps_hatt = ps_mic.tile([BPC, D], f32, tag="mic")
        for kc in range(KC):
            nc.tensor.matmul(
                ps_hatt,
                lhsT=hT_all[:, kc, :],
                rhs=w_all[:, kc, :],
                start=(kc == 0),
                stop=(kc == KC - 1),
            )
        b2_bc = consts.tile([BPC, D], f32)
        nc.gpsimd.dma_start(out=b2_bc, in_=bcast_p(b2_ap, BPC))
        hatt = consts.tile([BPC, D], f32)
        nc.vector.tensor_add(hatt, ps_hatt, b2_bc)

        # ---- main loop over this core's batches
        for b in range(BPC):
            # broadcast h_att[b] to 128 partitions on PE, downcast to bf16
            bc_ps = ps_bc.tile([128, D], f32, tag="bc")
            nc.tensor.matmul(
                bc_ps,
                lhsT=onehots[:, b * 128 : (b + 1) * 128],
                rhs=hatt,
                start=True,
                stop=True,
            )
            hb = hbpool.tile([128, D], bf16)
            nc.scalar.copy(hb, bc_ps)

            p_t = ppool.tile([128, RC, D], bf16, tag="p")
            nc.scalar.dma_start(
                out=p_t, in_=p_ap[b].rearrange("(p j) d -> p j d", p=128)
            )
            nc.vector.tensor_add(p_t, p_t, bcast_mid(hb, RC))
            nc.scalar.activation(p_t, p_t, AF.Tanh)
            scores = spool.tile([128, RC], f32, tag="scores")
            for j in range(RC):
                # out = (e * 1.0) * w_alpha; accum_out = sum -> score col
                nc.vector.scalar_tensor_tensor(
                    out=p_t[:, j, :],
                    in0=p_t[:, j, :],
                    scalar=1.0,
                    in1=walpha_bc,
                    op0=ALU.mult,
                    op1=ALU.mult,
                    accum_out=scores[:, j : j + 1],
                )

            # softmax over the 1024 scores laid out as [128 partitions, RC]
            m1 = spool.tile([128, 1], f32, tag="m1")
            nc.vector.reduce_max(m1, scores, axis=AX.X)
            mx = spool.tile([128, 1], f32, tag="mx")
            nc.gpsimd.partition_all_reduce(
                mx, m1, channels=128, reduce_op=bass_isa.ReduceOp.max
            )
            nm = spool.tile([128, 1], f32, tag="nm")
            nc.vector.tensor_scalar_mul(nm, mx, -1.0)
            expb = spool.tile([128, RC], f32, tag="expb")
            nc.scalar.activation(expb, scores, AF.Exp, bias=nm, scale=1.0)
            s1 = spool.tile([128, 1], f32, tag="s1")
            nc.vector.reduce_sum(s1, expb, axis=AX.X)
            sm = spool.tile([128, 1], f32, tag="sm")
            nc.gpsimd.partition_all_reduce(
                sm, s1, channels=128, reduce_op=bass_isa.ReduceOp.add
            )
            rec = spool.tile([128, 1], f32, tag="rec")
            nc.vector.reciprocal(rec, sm)
            wgt = spool.tile([128, RC], bf16, tag="wgt")
            nc.vector.tensor_scalar_mul(wgt, expb, rec)

            # ---- phase 2: out[b] = weight @ att_feats[b]
            a_t = apool.tile([128, RC, F], bf16, tag="a")
            nc.sync.dma_start(
                out=a_t, in_=att_ap[b].rearrange("(p j) f -> p j f", p=128)
            )
            ps0 = ps_mm.tile([1, 512], f32, tag="ps0")
            ps1 = ps_mm.tile([1, 512], f32, tag="ps1")
            for j in range(RC):
                nc.tensor.matmul(
                    ps0,
                    lhsT=wgt[:, j : j + 1],
                    rhs=a_t[:, j, 0:512],
                    start=(j == 0),
                    stop=(j == RC - 1),
                )
                nc.tensor.matmul(
                    ps1,
                    lhsT=wgt[:, j : j + 1],
                    rhs=a_t[:, j, 512:1024],
                    start=(j == 0),
                    stop=(j == RC - 1),
                )
            ob = outp.tile([1, F], f32)
            nc.scalar.copy(ob[:, 0:512], ps0)
            nc.scalar.copy(ob[:, 512:1024], ps1)
            nc.gpsimd.dma_start(out=out_ap[b : b + 1, :], in_=ob)

    nc.compile()
    return nc


def _get_program():
    if "nc" not in _cached:
        _cached["nc"] = _build_program()
    return _cached["nc"]


def _make_in_maps(inputs):
    import ml_dtypes

    bf = ml_dtypes.bfloat16
    h = np.ascontiguousarray(np.asarray(inputs["h"], dtype=np.float32))
    att = np.ascontiguousarray(np.asarray(inputs["att_feats"])).astype(bf)
    p = np.ascontiguousarray(np.asarray(inputs["p_att_feats"])).astype(bf)
    W = np.ascontiguousarray(np.asarray(inputs["W_h2att"], dtype=np.float32))
    b2 = np.ascontiguousarray(
        np.asarray(inputs["b_h2att"], dtype=np.float32).reshape(1, D)
    )
    wa = np.asarray(inputs["w_alpha"]).reshape(1, D).astype(bf)
    # b_alpha is a scalar added to every score; softmax is shift-invariant.
    onehots = np.ascontiguousarray(
        np.kron(np.eye(BPC, dtype=np.float32), np.ones((1, 128), dtype=np.float32))
    )
    in_maps = []
    for c in range(N_CORES):
        lo, hi = c * BPC, (c + 1) * BPC
        in_maps.append(
            {
                "onehots": onehots,
                "h": h[lo:hi],
                "att": att[lo:hi],
                "p": p[lo:hi],
                "w_h2att": W,
                "b_h2att": b2,
                "w_alpha": wa,
            }
        )
    return in_maps


def kernel(**inputs) -> np.ndarray:
    from concourse.bass_utils import run_bass_kernel_spmd

    nc = _get_program()
    in_maps = _make_in_maps(inputs)
    res = run_bass_kernel_spmd(nc, in_maps, list(range(N_CORES)))
    out = np.concatenate([res.results[c]["out"] for c in range(N_CORES)], axis=0)
    return out.astype(np.float32)
